# revision 53
# baseline (speedup 1.0000x reference)
"""Multi-head attention kernel for 8 Trainium2 NeuronCores (Bass/Tile).

Problem: B=2, L=2048, D=1024, H=16 heads, DK=64.
Sharding: core c -> batch b = c//4, head-group g = c%4 (4 heads each).
Each core computes its 4 heads' attention + its slice of the output
projection; the host sums the 4 partial outputs per batch (exact, since
Out = sum_g C_g @ Wo_g) and adds the bo / bv-derived bias terms.

Layout strategy (everything flows transposed so no on-chip transposes
are ever needed):
  - host supplies X^T [D, L] per input (bf16)
  - Q^T, K^T [256, L] produced directly (lhsT = W natural, rhs = X^T)
  - S^T[j,i] tiles via lhsT=K^T, rhs=Q^T; two heads packed per matmul
    round via PE row-tiling (K=64 -> row positions 0 and 64)
  - P~ = exp(S^T * scale) on ACT, PSUM -> SBUF (no max subtraction:
    |S*scale| <= ~3 for these input stats)
  - C~^T[dk,i] += V[j,dk].T-free matmul with a concurrent M=1 ones
    matmul (col position 64) accumulating the softmax denominators
  - normalization after the fact: recip(sums) broadcast across 64
    partitions with a K=1 bf16 matmul, then one DVE multiply
  - O[i,:] via lhsT=C^T, rhs=Wo natural; DMA out per 128-row tile

Schedule (v2): the 128-step exp stream is the clock (ACT ~1.11us/step).
Everything else is threaded through its PE/DVE slack:
  - DMA order: w+biases, xk blocks, xq block 0, wv, xv j-tiles, xq 1-3
  - K/Q0 projections run per column block as DMA lands (phase 1)
  - V projected just-in-time per j-tile inside it-block 0's steps
  - Q(it1..3) projected 1 matmul/step inside earlier blocks' steps
  - O(it-1) matmuls spread 1/step instead of an 8-matmul burst
  - norm broadcast matmuls in bf16 (fp32 was 5x slower on PE)
  - output written bf16; host sums partials in f32
"""

import sys

sys.path.insert(0, "/opt/trn_rl_repo")

from contextlib import ExitStack

import ml_dtypes
import numpy as np

import concourse.bass as bass
import concourse.tile as tile
from concourse import bacc, mybir
from concourse.bass_utils import run_bass_kernel_spmd


def _install_ntff_hook_shim():
    """The agent image's ``antenv`` lacks ``axon_hooks``, so the boot shim
    silently skips NTFF-profile-hook registration and ``run_bass_kernel_spmd``
    crashes on import when BASS_TRACE=1. Provide the module and register the
    ctypes hook ourselves; degrade to no-tracing on any failure."""
    import types

    if "antenv.axon_hooks" in sys.modules:
        return
    mod = types.ModuleType("antenv.axon_hooks")
    mod._hook = None
    mod.set_axon_ntff_profile_hook = lambda h: setattr(mod, "_hook", h)
    mod.get_axon_ntff_profile_hook = lambda: mod._hook
    sys.modules["antenv.axon_hooks"] = mod
    try:
        import antenv

        antenv.axon_hooks = mod
    except Exception:
        pass
    try:
        from trn_agent_boot.trn_boot import _ntff_profile_via_ctypes

        mod._hook = _ntff_profile_via_ctypes("/opt/axon/libaxon_pjrt.so")
    except Exception:
        pass


_install_ntff_hook_shim()

B, L, D, H, DK = 2, 2048, 1024, 16, 64
NCORES = 8
GROUPS = 4  # head-groups == cores per batch
NH = H // GROUPS  # 4 heads per core
CG = NH * DK  # 256 projected features per core
DT = D // 128  # 8 contraction tiles
CT = CG // 128  # 2 c-tiles
IT = L // 512  # 4 query blocks of 512
LT = L // 128  # 16 key/query tiles of 128
SCALE = 1.0 / float(np.sqrt(DK))

F32 = mybir.dt.float32
BF16 = mybir.dt.bfloat16
Identity = mybir.ActivationFunctionType.Identity
Exp = mybir.ActivationFunctionType.Exp

_built = None
_last_results = None


def _build():
    nc = bacc.Bacc()

    xq_d = nc.dram_tensor("xq_t", [D, L], BF16, kind="ExternalInput")
    xk_d = nc.dram_tensor("xk_t", [D, L], BF16, kind="ExternalInput")
    xv_d = nc.dram_tensor("xv_t", [D, L], BF16, kind="ExternalInput")
    wq_d = nc.dram_tensor("wq", [D, CG], BF16, kind="ExternalInput")
    wk_d = nc.dram_tensor("wk", [D, CG], BF16, kind="ExternalInput")
    wv_d = nc.dram_tensor("wv", [D, CG], BF16, kind="ExternalInput")
    wo_d = nc.dram_tensor("wo", [CG, D], BF16, kind="ExternalInput")
    bq_d = nc.dram_tensor("bq", [CG], F32, kind="ExternalInput")
    bk_d = nc.dram_tensor("bk", [CG], F32, kind="ExternalInput")
    out_d = nc.dram_tensor("out_p", [L, D], BF16, kind="ExternalOutput")

    with ExitStack() as ctx:
        tc = ctx.enter_context(tile.TileContext(nc))
        const = ctx.enter_context(tc.tile_pool(name="const", bufs=1))
        xp = ctx.enter_context(tc.tile_pool(name="xp", bufs=3))
        wp = ctx.enter_context(tc.tile_pool(name="wp", bufs=1))
        proj = ctx.enter_context(tc.tile_pool(name="proj", bufs=1))
        pp = ctx.enter_context(tc.tile_pool(name="pp", bufs=4))
        op_ = ctx.enter_context(tc.tile_pool(name="op", bufs=3))
        st = ctx.enter_context(tc.tile_pool(name="st", bufs=2))

        ones64 = const.tile([128, 64], BF16)
        nc.vector.memset(ones64, 1.0)
        # touch the Exp activation table once, long before the first real
        # exp, so the 1.3us table load happens during the DMA wait
        tbl = const.tile([1, 8], F32, name="tbl")
        nc.scalar.activation(out=tbl, in_=ones64[0:1, 0:8], func=Exp, scale=1.0)

        # ---------------- DMA issue order (one Sync queue, 8 HW queues) ----
        wq_sb = wp.tile([128, DT, CG], BF16, tag="wq")
        wk_sb = wp.tile([128, DT, CG], BF16, tag="wk")
        wv_sb = wp.tile([128, DT, CG], BF16, tag="wv")
        wo_sb = wp.tile([128, CT, D], BF16, tag="wo")
        bq_sb = wp.tile([128, CT], F32, tag="bq")
        bk_sb = wp.tile([128, CT], F32, tag="bk")

        nc.sync.dma_start(out=wk_sb, in_=wk_d[:, :].rearrange("(t p) c -> p t c", p=128))

        # x tiles arrive as column blocks so projections can chase the DMA
        xk_sb = xp.tile([128, DT, L], BF16, tag="x", name="xk_sb")
        xq_sb = xp.tile([128, DT, L], BF16, tag="x", name="xq_sb")
        xv_sb = xp.tile([128, DT, L], BF16, tag="x", name="xv_sb")

        def load_cols(x_sb, x_d, c0, c1):
            nc.sync.dma_start(
                out=x_sb[:, :, c0:c1],
                in_=x_d[:, c0:c1].rearrange("(t p) c -> p t c", p=128),
            )

        # Sync carries only low-descriptor-count loads (weights + xk row
        # d-tiles, 128 descriptors each).  All column-block loads (1024
        # descriptors — the Sync hardware DGE generates those at ~3.5ns each
        # and would serialize for 3-6us per issue) go through the GpSimd
        # software DGE, which emits them in ~1.1us and is otherwise idle.
        def load_cols_gp(x_sb, x_d, c0, c1):
            nc.gpsimd.dma_start(
                out=x_sb[:, :, c0:c1],
                in_=x_d[:, c0:c1].rearrange("(t p) c -> p t c", p=128),
            )

        # xk immediately after wk: the K projection gates the first exp, so
        # nothing else may run ahead of it on the HBM.
        for dt in range(DT):
            nc.sync.dma_start(
                out=xk_sb[:, dt, :], in_=xk_d[dt * 128 : (dt + 1) * 128, :]
            )
        nc.sync.dma_start(out=bk_sb, in_=bk_d[:].rearrange("(c p) -> p c", p=128))
        nc.sync.dma_start(out=wq_sb, in_=wq_d[:, :].rearrange("(t p) c -> p t c", p=128))
        nc.sync.dma_start(out=bq_sb, in_=bq_d[:].rearrange("(c p) -> p c", p=128))
        load_cols(xq_sb, xq_d, 0, 512)
        nc.sync.dma_start(out=wv_sb, in_=wv_d[:, :].rearrange("(t p) c -> p t c", p=128))
        for b in range(IT):
            load_cols(xv_sb, xv_d, b * 512, (b + 1) * 512)
        for b in range(1, IT):
            load_cols(xq_sb, xq_d, b * 512, (b + 1) * 512)
        nc.sync.dma_start(out=wo_sb, in_=wo_d[:, :].rearrange("(t p) d -> p t d", p=128))

        # ---------------- phase 1: K and Q0 projections ----------------
        kT = [proj.tile([128, L], BF16, tag=f"kT{ct}", name=f"kT{ct}") for ct in range(CT)]
        qT = [proj.tile([128, L], BF16, tag=f"qT{ct}", name=f"qT{ct}") for ct in range(CT)]
        cT = [proj.tile([128, L], BF16, tag=f"cT{ct}", name=f"cT{ct}") for ct in range(CT)]
        # v_sb holds [V_h | 1] blocks of 65 columns per head: the ones
        # column makes the C~ matmul also accumulate the softmax
        # denominator in psum row 64 (M=65 costs the same as M=64).
        v_sb = proj.tile([128, LT, NH * 65], BF16, tag="v")
        nc.vector.memset(
            v_sb.rearrange("p l (h c) -> p l h c", h=NH)[:, :, :, 64:65], 1.0
        )

        ph1 = ExitStack()
        ps1 = ph1.enter_context(tc.tile_pool(name="ps1", bufs=8, space="PSUM"))

        # PE warmup: a run of tiny matmuls keeps the PE busy-streak alive so
        # the p-state ramp is done before the first real projection.
        warm_ps = ps1.tile([64, 64], F32, tag="p1ps", name="warm_ps")
        for _ in range(80):
            nc.tensor.matmul(warm_ps, lhsT=ones64[0:1, :], rhs=ones64[0:1, :],
                             start=True, stop=True)

        def proj_block(x_sb, w_sb, b_sb, out_t, ct, blk, name):
            t_ps = ps1.tile([128, 512], F32, tag="p1ps", name=name)
            bsl = slice(blk * 512, (blk + 1) * 512)
            for dt in range(DT):
                nc.tensor.matmul(
                    t_ps,
                    lhsT=w_sb[:, dt, ct * 128 : (ct + 1) * 128],
                    rhs=x_sb[:, dt, bsl],
                    start=(dt == 0),
                    stop=(dt == DT - 1),
                )
            nc.scalar.activation(
                out=out_t[ct][:, bsl], in_=t_ps, func=Identity,
                bias=b_sb[:, ct : ct + 1], scale=1.0,
            )

        # Q block 0 first (its column-block DMA lands before the xk rows
        # finish), then K accumulating d-tile by d-tile into 8
        # simultaneously-live psums, chasing the row DMAs; K drains
        # alternate ACT/DVE so neither serializes the phase tail.
        def vproj_body(v_ps, lt):
            for dt in range(DT):
                nc.tensor.matmul(
                    v_ps,
                    lhsT=xv_sb[:, dt, lt * 128 : (lt + 1) * 128],
                    rhs=wv_sb[:, dt, :],
                    start=(dt == 0),
                    stop=(dt == DT - 1),
                )
            nc.vector.tensor_copy(
                out=v_sb[:, lt, :].rearrange("p (h c) -> p h c", h=NH)[:, :, 0:64],
                in_=v_ps.rearrange("p (h c) -> p h c", h=NH),
            )

        k_pss = [
            ps1.tile([128, 512], F32, tag="p1ps", name=f"kps{g}") for g in range(8)
        ]
        for dt in range(DT):
            for g in range(8):
                ct, blk = g // IT, g % IT
                nc.tensor.matmul(
                    k_pss[g],
                    lhsT=wk_sb[:, dt, ct * 128 : (ct + 1) * 128],
                    rhs=xk_sb[:, dt, blk * 512 : (blk + 1) * 512],
                    start=(dt == 0),
                    stop=(dt == DT - 1),
                )
        for g in range(8):
            ct, blk = g // IT, g % IT
            bsl = slice(blk * 512, (blk + 1) * 512)
            if g % 2 == 0:
                nc.scalar.activation(
                    out=kT[ct][:, bsl], in_=k_pss[g], func=Identity,
                    bias=bk_sb[:, ct : ct + 1], scale=1.0,
                )
            else:
                nc.vector.tensor_scalar_add(
                    out=kT[ct][:, bsl], in0=k_pss[g], scalar1=bk_sb[:, ct : ct + 1]
                )
        proj_block(xq_sb, wq_sb, bq_sb, qT, 0, 0, "qps0_0")
        proj_block(xq_sb, wq_sb, bq_sb, qT, 1, 0, "qps1_0")
        ph1.close()  # release phase-1 PSUM banks

        # ---------------- attention pools ----------------
        pss = ctx.enter_context(tc.tile_pool(name="pss", bufs=2, space="PSUM"))
        psc = ctx.enter_context(tc.tile_pool(name="psc", bufs=2, space="PSUM"))
        pst = ctx.enter_context(tc.tile_pool(name="pst", bufs=2, space="PSUM"))
        strip = st.tile([97, L], BF16, tag="strip", bufs=1)

        # ---- deferred work, injected into the step stream ----
        def vproj(lt):
            # V j-tile lt: 8 accumulating matmuls + one strided DVE drain
            vproj_body(
                pst.tile([128, CG], F32, tag="t512", name=f"v_ps{lt}"), lt
            )

        qproj_state = {}

        def qproj_mm(it, ct, dt):
            # one matmul of the deferred Q(it) projection for 512-block `it`
            key = (it, ct)
            if dt == 0:
                qproj_state[key] = ps_q = pst.tile(
                    [128, 512], F32, tag="t512", name=f"q_ps{it}_{ct}"
                )
            else:
                ps_q = qproj_state[key]
            bsl = slice(it * 512, (it + 1) * 512)
            nc.tensor.matmul(
                ps_q,
                lhsT=wq_sb[:, dt, ct * 128 : (ct + 1) * 128],
                rhs=xq_sb[:, dt, bsl],
                start=(dt == 0),
                stop=(dt == DT - 1),
            )
            if dt == DT - 1:
                # drain on DVE — an ACT drain would stall the exp stream
                nc.vector.tensor_scalar_add(
                    out=qT[ct][:, bsl], in0=ps_q, scalar1=bq_sb[:, ct : ct + 1]
                )
                del qproj_state[key]

        # ---- flat attention pipeline over (it, hp, jt) steps ----
        steps = [
            (it, hp, jt) for it in range(IT) for hp in range(2) for jt in range(LT)
        ]
        cps_map = {}
        stage_map = {}

        def emit_S(it, hp, jt):
            isl = slice(it * 512, (it + 1) * 512)
            s_ps = pss.tile([128, 1024], F32, tag="sps", name="s_ps")
            for hl in range(2):
                rsl = slice(64 * hl, 64 * hl + 64)
                nc.tensor.matmul(
                    s_ps[:, hl * 512 : (hl + 1) * 512],
                    lhsT=kT[hp][rsl, jt * 128 : (jt + 1) * 128],
                    rhs=qT[hp][rsl, isl],
                    start=True,
                    stop=True,
                )
            p_t = pp.tile([128, 1024], BF16, tag="pt", name="p_t")
            nc.scalar.activation(out=p_t, in_=s_ps, func=Exp, scale=SCALE)
            return p_t

        def emit_C(it, hp, jt, p_t):
            isl = slice(it * 512, (it + 1) * 512)
            if jt == 0:
                cps_map[(it, hp)] = [
                    psc.tile([65, 512], F32, tag="cps", name=f"cps{hl}")
                    for hl in range(2)
                ]
            cps = cps_map[(it, hp)]
            for hl in range(2):
                h = 2 * hp + hl
                nc.tensor.matmul(
                    cps[hl],
                    lhsT=v_sb[:, jt, 65 * h : 65 * h + 65],
                    rhs=p_t[:, hl * 512 : (hl + 1) * 512],
                    start=(jt == 0),
                    stop=(jt == LT - 1),
                )
            if jt == LT - 1:
                stage = st.tile([65, 1024], F32, tag="stage", name="stage")
                stage_map[(it, hp)] = stage
                tail = (it, hp) == (IT - 1, 1)

                def stage_cp(hl):
                    nc.vector.tensor_copy(
                        out=stage[64:65, hl * 512 : (hl + 1) * 512],
                        in_=cps[hl][64:65, :],
                    )

                def ct_cp(hl, act=False):
                    if act:
                        # idle post-stream ACT runs one drain in parallel
                        nc.scalar.activation(
                            out=cT[hp][64 * hl : 64 * hl + 64, isl],
                            in_=cps[hl][0:64, :], func=Identity, scale=1.0,
                        )
                    else:
                        nc.vector.tensor_copy(
                            out=cT[hp][64 * hl : 64 * hl + 64, isl],
                            in_=cps[hl][0:64, :],
                        )

                # stage rows first: they gate the norm round-trip, and the
                # deferred first-C of the next half-block tolerates the
                # later psum release
                stage_cp(0); stage_cp(1); ct_cp(0); ct_cp(1, act=tail)

        def emit_norm_dma(it, hp):
            # denominators -> 128-partition layout -> reciprocal -> strip rows
            stage = stage_map.pop((it, hp))
            isl = slice(it * 512, (it + 1) * 512)
            sq = st.tile([128, 8], F32, tag="sq")
            sq2 = st.tile([128, 8], F32, tag="sq2")
            sq2b = st.tile([128, 8], BF16, tag="sq2b")
            nc.sync.dma_start(out=sq[:, :], in_=stage[64:65, :])
            nc.vector.reciprocal(out=sq2, in_=sq)
            nc.vector.tensor_copy(out=sq2b, in_=sq2)
            for hl in range(2):
                h = 2 * hp + hl
                nc.sync.dma_start(
                    out=strip[32 * h : 32 * h + 1, isl],
                    in_=sq2b[64 * hl : 64 * hl + 64, :],
                )

        norm_ps = {}

        def emit_norm_mm(it, hp, hls=(0, 1)):
            isl = slice(it * 512, (it + 1) * 512)
            if 0 in hls:
                norm_ps[(it, hp)] = pst.tile([128, 512], F32, tag="t512", name="n_ps")
            n_ps = norm_ps[(it, hp)]
            for hl in hls:
                h = 2 * hp + hl
                nc.tensor.matmul(
                    n_ps[64 * hl : 64 * hl + 64, :],
                    lhsT=ones64[32 * h : 32 * h + 1, :],
                    rhs=strip[32 * h : 32 * h + 1, isl],
                    start=True,
                    stop=True,
                    tile_position=(32 * h, 64 * hl),
                )
                rsl = slice(64 * hl, 64 * hl + 64)
                nc.vector.tensor_mul(
                    out=cT[hp][rsl, isl],
                    in0=cT[hp][rsl, isl],
                    in1=n_ps[rsl, :],
                )
            if 1 in hls:
                del norm_ps[(it, hp)]

        # O(it) is unrolled into 16 single matmuls, injected 1/step.
        o_state = {}

        def o_mm(it, s, dn, ct):
            # slice s of block it, output-column half dn, contraction tile ct
            i0 = it * 512 + s * 128
            key = (it, s)
            if dn == 0 and ct == 0:
                o_state[key] = op_.tile([128, D], BF16, tag="osb", name=f"osb{it}_{s}")
            if ct == 0:
                o_state[key, "ps"] = pst.tile(
                    [128, 512], F32, tag="t512", name=f"o_ps{it}_{s}_{dn}"
                )
            o_ps = o_state[key, "ps"]
            nc.tensor.matmul(
                o_ps,
                lhsT=cT[ct][:, i0 : i0 + 128],
                rhs=wo_sb[:, ct, dn * 512 : (dn + 1) * 512],
                start=(ct == 0),
                stop=(ct == CT - 1),
            )
            if ct == CT - 1:
                o_sb = o_state[key]
                nc.vector.tensor_copy(
                    out=o_sb[:, dn * 512 : (dn + 1) * 512], in_=o_ps
                )
                del o_state[key, "ps"]
                if dn == 1:
                    nc.sync.dma_start(out=out_d[i0 : i0 + 128, :], in_=o_sb)
                    del o_state[key]

        def o_ops(it):
            return [
                (it, s, dn, ct) for s in range(4) for dn in range(2) for ct in range(CT)
            ]

        # ---- static injection schedule ----
        # inj[step_index] = list of thunks run before that step's S matmul.
        inj = [[] for _ in range(len(steps) + 1)]

        def sidx(it, hp, jt):
            return it * 2 * LT + hp * LT + jt

        # V j-tiles: exactly one projection per step — C(lt) lands at step
        # lt+1 under the first-C deferral, so vproj(lt) at step lt leads it
        # by a full emission slot while spreading the xv-arrival wait.
        inj[0].append(lambda: vproj(0))
        for lt in range(1, LT):
            inj[sidx(0, 0, lt)].append(lambda lt=lt: vproj(lt))

        # Deferred Q projections: 1 matmul/step, at most one injected matmul
        # per step (jt15 doubles dt6/dt7 to dodge the jt8 norm matmuls).
        #   Q(it): ct0 on (it-1,0) jt0..7, ct1 on (it-1,1) jt9..15.
        # qT[ct1] of block `it` is first read at (it,1,0), 16 steps after
        # its drain, so the late ct1 placement is safe.
        qslots = {}
        for it in range(1, IT):
            qslots[it] = (
                [(it - 1, 0, jt) for jt in range(8)]
                + [(it - 1, 1, jt) for jt in (9, 10, 11, 12, 13, 14, 15, 15)]
            )
        # Q(1) is needed at step 32, before (0,1) ends: keep its ct0 on
        # (0,1) jt0..7 instead (it0/hp0 is saturated by V projections).
        qslots[1] = (
            [(0, 1, jt) for jt in range(8)]
            + [(0, 1, jt) for jt in (9, 10, 11, 12, 13, 14, 15, 15)]
        )
        for it, slots_q in qslots.items():
            for m, (bit, bhp, bjt) in enumerate(slots_q):
                ct, dt = m // 8, m % 8
                inj[sidx(bit, bhp, bjt)].append(
                    lambda it=it, ct=ct, dt=dt: qproj_mm(it, ct, dt)
                )

        # norm chains: broadcast matmuls split across two steps so no single
        # step carries a 2-matmul norm burst.
        for it in range(IT):
            inj[sidx(it, 1, 2)].append(lambda it=it: emit_norm_dma(it, 0))
            inj[sidx(it, 1, 8)].append(lambda it=it: emit_norm_mm(it, 0, (0,)))
            inj[sidx(it, 1, 9)].append(lambda it=it: emit_norm_mm(it, 0, (1,)))
            if it > 0:
                inj[sidx(it, 0, 2)].append(lambda it=it: emit_norm_dma(it - 1, 1))
                inj[sidx(it, 0, 8)].append(lambda it=it: emit_norm_mm(it - 1, 1, (0,)))
                inj[sidx(it, 0, 9)].append(lambda it=it: emit_norm_mm(it - 1, 1, (1,)))

        # O(it-1): 16 matmuls spread 1/step, dodging the norm steps.
        for it in range(1, IT):
            ops = o_ops(it - 1)
            slots = (
                [(0, jt) for jt in range(10, LT)]
                + [(1, jt) for jt in range(8)]
                + [(1, 7), (1, 8)]
            )
            for (hp, jt), op in zip(slots, ops):
                inj[sidx(it, hp, jt)].append(
                    lambda op=op: o_mm(*op)
                )

        # C normally lags S by one step.  The first C of each half-block is
        # held one extra step (emitted with jt1's at jt2) so the fresh C
        # psum's WAR on the previous half-block's drain copies is off the
        # critical path.
        pend = []
        for n, (it, hp, jt) in enumerate(steps):
            # stream-critical S (and C) go first; injected norm/O/Q/V
            # matmuls fill the PE's exp-wait instead of delaying S
            p_t = emit_S(it, hp, jt)
            pend.append((it, hp, jt, p_t))
            keep = 2 if jt == 1 else 1
            while len(pend) > keep:
                emit_C(*pend.pop(0))
            for thunk in inj[n]:
                thunk()
        while pend:
            emit_C(*pend.pop(0))

        # ---- tail: final norm + O(3), minimum critical path ----
        emit_norm_dma(IT - 1, 1)
        # keep the PE busy-streak alive through the norm DMA round-trip so
        # the O matmuls run at full clock instead of the mid p-state
        fill_ps = pss.tile([128, 1024], F32, tag="sps", name="fill_ps")
        for _ in range(24):
            nc.tensor.matmul(fill_ps[0:64, 0:512], lhsT=ones64[0:1, :],
                             rhs=cT[0][0:1, 0:512], start=True, stop=True)

        # O(3) in the (now free) S psum banks.  The ct=0 accumulations of the
        # first two slices only read cT[0] (normalized a block ago), so they
        # run during the final norm round-trip; their ct=1 halves follow the
        # norm multiplies.
        def tail_o_mms(o_ps, i0, ct):
            for dn in range(2):
                nc.tensor.matmul(
                    o_ps[:, dn * 512 : (dn + 1) * 512],
                    lhsT=cT[ct][:, i0 : i0 + 128],
                    rhs=wo_sb[:, ct, dn * 512 : (dn + 1) * 512],
                    start=(ct == 0),
                    stop=(ct == CT - 1),
                )

        def tail_o_drain(o_ps, i0, s):
            # alternate the big psum->bf16 drains across DVE and the (idle
            # post-stream) ACT engine so they run pairwise in parallel
            o_sb = op_.tile([128, D], BF16, tag="osb", name=f"tosb{s}")
            if s % 2 == 0:
                nc.vector.tensor_copy(out=o_sb, in_=o_ps)
            else:
                nc.scalar.activation(out=o_sb, in_=o_ps, func=Identity, scale=1.0)
            nc.sync.dma_start(out=out_d[i0 : i0 + 128, :], in_=o_sb)

        base = (IT - 1) * 512
        tail_ps = {}
        for s in (0, 1):
            tail_ps[s] = pss.tile([128, 1024], F32, tag="sps", name=f"to_ps{s}")
            tail_o_mms(tail_ps[s], base + s * 128, 0)
        emit_norm_mm(IT - 1, 1)
        for s in (0, 1):
            tail_o_mms(tail_ps[s], base + s * 128, 1)
            tail_o_drain(tail_ps[s], base + s * 128, s)
        for s in (2, 3):
            o_ps = pss.tile([128, 1024], F32, tag="sps", name=f"to_ps{s}")
            for ct in range(CT):
                tail_o_mms(o_ps, base + s * 128, ct)
            tail_o_drain(o_ps, base + s * 128, s)

    nc.compile()
    return nc


def _get_built():
    global _built
    if _built is None:
        _built = _build()
    return _built


def _make_in_maps(query, key, value, Wq, bq, Wk, bk, Wv, bv, Wo, bo):
    bf = ml_dtypes.bfloat16
    xt = {}
    for b in range(B):
        xt[b] = {
            "xq_t": np.ascontiguousarray(query[b].T).astype(bf),
            "xk_t": np.ascontiguousarray(key[b].T).astype(bf),
            "xv_t": np.ascontiguousarray(value[b].T).astype(bf),
        }
    in_maps = []
    for c in range(NCORES):
        b, g = c // GROUPS, c % GROUPS
        cols = slice(g * CG, (g + 1) * CG)
        in_maps.append(
            {
                **xt[b],
                "wq": np.ascontiguousarray(Wq[:, cols]).astype(bf),
                "wk": np.ascontiguousarray(Wk[:, cols]).astype(bf),
                "wv": np.ascontiguousarray(Wv[:, cols]).astype(bf),
                "wo": np.ascontiguousarray(Wo[cols, :]).astype(bf),
                "bq": np.ascontiguousarray(bq[cols], dtype=np.float32),
                "bk": np.ascontiguousarray(bk[cols], dtype=np.float32),
            }
        )
    return in_maps


def kernel(query, key, value, Wq, bq, Wk, bk, Wv, bv, Wo, bo):
    global _last_results
    query = np.asarray(query, dtype=np.float32)
    key = np.asarray(key, dtype=np.float32)
    value = np.asarray(value, dtype=np.float32)
    Wq, Wk, Wv, Wo = (np.asarray(w, dtype=np.float32) for w in (Wq, Wk, Wv, Wo))
    bq, bk, bv, bo = (np.asarray(v, dtype=np.float32) for v in (bq, bk, bv, bo))

    nc = _get_built()
    in_maps = _make_in_maps(query, key, value, Wq, bq, Wk, bk, Wv, bv, Wo, bo)
    res = run_bass_kernel_spmd(nc, in_maps, core_ids=list(range(NCORES)))
    _last_results = res

    # bv contributes exactly bv @ Wo to every output row (softmax rows sum
    # to 1); bo is the plain output bias.
    bias = (bv @ Wo + bo).astype(np.float32)
    out = np.empty((B, L, D), dtype=np.float32)
    for b in range(B):
        acc = np.zeros((L, D), dtype=np.float32)
        for g in range(GROUPS):
            acc += res.results[b * GROUPS + g]["out_p"].astype(np.float32)
        out[b] = acc + bias
    return out
